# revision 25
# baseline (speedup 1.0000x reference)
"""Trainium2 Bass kernel for nn_AttentionMap (B=4, H=16, S=2048, d=64, rel_d=32).

out[b,h,q,k] = softmax_k( clip(Q)·clip(K)^T * d^-.5 + clip(PQ)·clip(PK)^T * rd^-.5 )

Strategy (measured ~298-300us HW vs 378-437us baseline):
  - Shard the 64 (b,h) slices across 8 NeuronCores, 8 per core (data parallel,
    no collectives; softmax is over the local k axis).
  - Host-side prep (pure layout): concat [q|pos_q] and [k|pos_k] along the
    feature dim (64+32=96) and transpose each (b,h) slice to [96, S] so both
    matmul operands arrive in [contraction, free] layout. All math (clamp,
    scale, matmul, softmax) runs on device.
  - Per (b,h): clamp to [-5,5] on VectorE writing the fp32r matmul operands
    directly; the relative-position scale ratio (sqrt2) is folded into the
    pos-QUERY rows (off the k-load critical path) and the common d^-.5 score
    scale into the ScalarE activation's `scale` argument: no standalone
    scale passes.
  - Per 128-row q tile: 4 fp32r matmuls into a [128,2048] PSUM tile (2-deep
    ring = all 8 banks), one ScalarE Exp over the tile writing BF16 directly
    with f32 accum_out row sums, VectorE reciprocal + per-row scale in bf16
    (4x DVE mode), two half-tile bf16 DMAs out.  Output is bf16 on the wire
    (tolerance 2e-2; bf16 adds ~2e-3), halving the output-DMA floor; the
    host upcasts to f32.
  - Engine balance (per-core busy): ScalarE exp is the wall at ~265us
    (1 elem/cycle/lane @1.2GHz is architectural); PE sits just under at
    ~264us (4 matmuls/tile, first each burst at cold p-state). Every 3rd
    tile's row-sum moves to a DVE tensor_reduce to shave the ScalarE
    accumulator-read aux op; pushing more tips DVE over the wall.
  - First/last tile run exp/normalize/DMA in column halves (two full-slot
    PSUM tiles each written only in their first half give each half-exp a
    dependency on exactly its own two matmuls): trims pipeline head/tail.
  - Next-bh input loads are emitted mid-tile-loop, one chunk per tile
    iteration (c=2..9), so the SP sequencer's dma_start issues (~0.65us
    each) never burst-block the per-tile output-DMA issues.
"""
import numpy as np
from contextlib import ExitStack

import concourse.tile as tile
from concourse import bacc, mybir
from concourse._compat import with_exitstack
from concourse.bass_utils import run_bass_kernel_spmd

F32 = mybir.dt.float32
F32R = mybir.dt.float32r
BF16 = mybir.dt.bfloat16

N_CORES = 8
B, H, S = 4, 16, 2048
DQ, DP = 64, 32
D = DQ + DP
SCALE = DQ ** -0.5
REL_SCALE = DP ** -0.5
POS_RATIO = REL_SCALE / SCALE  # = sqrt(2): folded into the pos-key operand
CLAMP = 5.0

# "bf16" (bf16 output wire format, exp scale folded, ~2e-3 err)
# "f32r" (f32 output wire format, ~2.5e-4 err)
MODE = "bf16"


@with_exitstack
def _attn_kernel(ctx: ExitStack, tc: tile.TileContext, out_d, qt_d, kt_d,
                 mode: str, n_bh: int, s: int):
    nc = tc.nc
    n_ct = s // 128          # q tiles per bh
    n_kb = s // 512          # 512-wide k blocks per psum tile

    stage = ctx.enter_context(tc.tile_pool(name="stage", bufs=3))
    opnd = ctx.enter_context(tc.tile_pool(name="opnd", bufs=3))
    expp = ctx.enter_context(tc.tile_pool(name="expp", bufs=10))
    small = ctx.enter_context(tc.tile_pool(name="small", bufs=16))
    scps = ctx.enter_context(tc.tile_pool(name="scps", bufs=2, space="PSUM"))

    out_dt = BF16 if mode == "bf16" else F32

    def prep(bh, first=False):
        """Load + clamp one bh's operands; returns (qT, kT) fp32r tiles.
        Column-chunked so compute can start before the full DMA lands; the
        first bh additionally front-loads a tiny q[:, :128] chunk (all the
        q columns tile 0's matmuls need) ahead of the k stream."""
        qs = stage.tile([D, s], F32, tag="qs")
        ks = stage.tile([D, s], F32, tag="ks")
        qT = opnd.tile([D, s], F32R, tag="qT")
        kT = opnd.tile([D, s], F32R, tag="kT")

        def load_q(h):
            nc.sync.dma_start(out=qs[:, h], in_=qt_d[bh, :, h])
            nc.vector.tensor_scalar(out=qT[:, h], in0=qs[:, h],
                                    scalar1=CLAMP, scalar2=-CLAMP,
                                    op0=mybir.AluOpType.min,
                                    op1=mybir.AluOpType.max)
            if mode == "bf16":
                # rel-pos scale ratio lives on the q side: q tiles are small
                # and off the k-load critical path
                nc.vector.tensor_scalar_mul(out=qT[DQ:, h], in0=qT[DQ:, h],
                                            scalar1=POS_RATIO)
            else:
                nc.vector.tensor_scalar_mul(out=qT[:DQ, h], in0=qT[:DQ, h],
                                            scalar1=SCALE)
                nc.vector.tensor_scalar_mul(out=qT[DQ:, h], in0=qT[DQ:, h],
                                            scalar1=REL_SCALE)

        def load_k(h):
            nc.sync.dma_start(out=ks[:, h], in_=kt_d[bh, :, h])
            nc.vector.tensor_scalar(out=kT[:, h], in0=ks[:, h],
                                    scalar1=CLAMP, scalar2=-CLAMP,
                                    op0=mybir.AluOpType.min,
                                    op1=mybir.AluOpType.max)

        if first:
            # head: the first exp needs k[:, :1024], q[:, :128] — issue
            # those first and in as few dma_starts as possible (SP issue
            # costs ~0.7us each, which dominates over transfer here)
            load_k(slice(0, s // 2))
            load_q(slice(0, 128))
            load_k(slice(s // 2, s))
            load_q(slice(128, 128 + (s - 128) // 2))
            load_q(slice(128 + (s - 128) // 2, s))
            return qT, kT
        # steady state: deferred single-chunk emitters so the SP sequencer
        # issues at most one prep dma_start per tile iteration instead of
        # bursting and stalling the per-tile output DMA issues
        emitters = [
            (lambda i=i: load_k(slice(i * (s // 4), (i + 1) * (s // 4))))
            for i in range(4)
        ] + [
            (lambda i=i: load_q(slice(i * (s // 4), (i + 1) * (s // 4))))
            for i in range(4)
        ]
        return qT, kT, emitters

    exp_scale = SCALE if mode == "bf16" else 1.0

    next_ops = prep(0, first=True)
    for bh in range(n_bh):
        if bh == 0:
            qT, kT = next_ops
        else:
            qT, kT = next_ops[0], next_ops[1]
        next_ops = None

        for c in range(n_ct):
            if bh + 1 < n_bh:
                # software-pipeline: emit the next bh's load/clamp early,
                # one input-DMA chunk per tile iteration (c = 2..9)
                if c == 2:
                    next_ops = prep(bh + 1)
                if next_ops is not None and 2 <= c < 2 + len(next_ops[2]):
                    next_ops[2][c - 2]()
            qcol = qT[:, c * 128:(c + 1) * 128]
            exp_sb = expp.tile([128, s], out_dt, tag="exp")
            rec = small.tile([128, 1], F32, tag="rec")
            tot = small.tile([128, 1], F32, tag="tot")
            rows = slice(c * 128, (c + 1) * 128)
            split = (bh == 0 and c == 0) or (bh == n_bh - 1 and c == n_ct - 1)
            if split:
                # first tile: exp per column half so ACT starts while the
                # head DMA streams in; last tile: so the final output DMA
                # starts earlier.  Two full-slot PSUM tiles, each written
                # only in its first half, give each half-exp a dependency
                # on exactly its own two matmuls.
                sca = scps.tile([128, s], F32, tag="sc", name="sca")
                scb = scps.tile([128, s], F32, tag="sc", name="scb")
                tot2 = small.tile([128, 2], F32, tag="tot2")
                for i, st in ((0, sca), (1, scb)):
                    for j in (0, 1):
                        nc.tensor.matmul(
                            st[:, j * 512:(j + 1) * 512], lhsT=qcol,
                            rhs=kT[:, (2 * i + j) * 512:(2 * i + j + 1) * 512],
                            start=True, stop=True)
                    h = slice(i * (s // 2), (i + 1) * (s // 2))
                    nc.scalar.activation(out=exp_sb[:, h], in_=st[:, :s // 2],
                                         func=mybir.ActivationFunctionType.Exp,
                                         scale=exp_scale,
                                         accum_out=tot2[:, i:i + 1])
                nc.vector.tensor_reduce(out=tot[:], in_=tot2[:],
                                        axis=mybir.AxisListType.X,
                                        op=mybir.AluOpType.add)
                nc.vector.reciprocal(out=rec[:], in_=tot[:])
                for i in (0, 1):
                    h = slice(i * (s // 2), (i + 1) * (s // 2))
                    nc.vector.tensor_scalar_mul(out=exp_sb[:, h],
                                                in0=exp_sb[:, h],
                                                scalar1=rec[:])
                    nc.sync.dma_start(out=out_d[bh, rows, h],
                                      in_=exp_sb[:, h])
                continue
            sc = scps.tile([128, s], F32, tag="sc")
            for j in range(n_kb):
                cols = slice(j * 512, (j + 1) * 512)
                nc.tensor.matmul(sc[:, cols],
                                 lhsT=qcol,
                                 rhs=kT[:, cols],
                                 start=True, stop=True)
            if c % 3 == 2:
                # every 3rd tile: row sums via a DVE reduce instead of the
                # ScalarE accumulator read — sheds ~60ns/tile off the ACT
                # critical path while keeping DVE below its own roofline
                nc.scalar.activation(out=exp_sb[:], in_=sc[:],
                                     func=mybir.ActivationFunctionType.Exp,
                                     scale=exp_scale)
                nc.vector.tensor_reduce(out=tot[:], in_=exp_sb[:],
                                        axis=mybir.AxisListType.X,
                                        op=mybir.AluOpType.add)
            else:
                nc.scalar.activation(out=exp_sb[:], in_=sc[:],
                                     func=mybir.ActivationFunctionType.Exp,
                                     scale=exp_scale, accum_out=tot[:])
            nc.vector.reciprocal(out=rec[:], in_=tot[:])
            nc.vector.tensor_scalar_mul(out=exp_sb[:], in0=exp_sb[:],
                                        scalar1=rec[:])
            # two half-tile output DMAs: drains the tail faster and keeps
            # per-queue bursts small so prep inputs interleave cleanly
            for i in range(2):
                cols = slice(i * (s // 2), (i + 1) * (s // 2))
                nc.sync.dma_start(out=out_d[bh, rows, cols],
                                  in_=exp_sb[:, cols])


def build(mode: str = MODE, n_bh: int = N_CORES, s: int = S):
    nc = bacc.Bacc("TRN2", target_bir_lowering=False, debug=False,
                   num_devices=N_CORES)
    out_dt = BF16 if mode == "bf16" else F32
    qt_d = nc.dram_tensor("qt", [n_bh, D, s], F32, kind="ExternalInput").ap()
    kt_d = nc.dram_tensor("kt", [n_bh, D, s], F32, kind="ExternalInput").ap()
    out_d = nc.dram_tensor("out", [n_bh, s, s], out_dt,
                           kind="ExternalOutput").ap()
    with tile.TileContext(nc) as tc:
        _attn_kernel(tc, out_d, qt_d, kt_d, mode, n_bh, s)
    nc.compile()
    return nc


def _host_prep(keys, queries, pos_key, pos_query):
    """[B,H,S,d] inputs -> per-core {'qt','kt'} slices in [bh, 96, S] layout."""
    qcat = np.concatenate([np.asarray(queries), np.asarray(pos_query)], axis=-1)
    kcat = np.concatenate([np.asarray(keys), np.asarray(pos_key)], axis=-1)
    qt = np.ascontiguousarray(
        qcat.reshape(B * H, S, D).swapaxes(1, 2), dtype=np.float32)
    kt = np.ascontiguousarray(
        kcat.reshape(B * H, S, D).swapaxes(1, 2), dtype=np.float32)
    per = (B * H) // N_CORES
    return [{"qt": qt[c * per:(c + 1) * per], "kt": kt[c * per:(c + 1) * per]}
            for c in range(N_CORES)]


def _run(keys, queries, pos_key, pos_query, mode=MODE, trace=False, **kw):
    in_maps = _host_prep(keys, queries, pos_key, pos_query)
    nc = build(mode=mode)
    res = run_bass_kernel_spmd(nc, in_maps, list(range(N_CORES)), trace=trace, **kw)
    out = np.concatenate(
        [np.asarray(res.results[c]["out"], dtype=np.float32)
         for c in range(N_CORES)], axis=0)
    return out.reshape(B, H, S, S), res


def kernel(keys, queries, pos_key, pos_query):
    out, _ = _run(keys, queries, pos_key, pos_query)
    return out
